# revision 33
# baseline (speedup 1.0000x reference)
"""Trainium2 Bass kernel for a two-window sparse causal self-attention block.

Model (B=2, T=2048, C=1024):
  - 8 "short" heads: d_qk=32,  window 256
  - 8 "long"  heads: d_qk=128, window 1024
  - value/output head dim 64, output projection C x C.

Sharding (8 cores): data-parallel over batch (2) x head-parallel over head
groups (4). Core c = 4*b + g handles batch b and heads {2g, 2g+1} of both the
short and long sets. Each core computes its 4 heads' attention plus the
corresponding 256 rows of Wproj, producing a partial [T, C] output; the host
sums the 4 partials per batch element.

Device-side design notes (v2, software-pipelined):
  - scores in f32r (full PE rate at N=512, exact fp32 bits); p/v/Wproj/y in
    bf16 (any-N full rate, 2x DVE modes). Softmax sums kept exact in fp32
    PSUM via a bf16 ones column appended to v.
  - everything computed transposed so no on-device transposes: host passes
    xT [C, T]; projections give qT/kT [d, T] and v [T, dv]; sT[k, q] =
    kT.T @ qT; yT[dv, q] = v_aug.T @ pT.
  - the projection work is software-pipelined into the attention loop:
    projection chunk tch (512 tokens) is emitted between the attention ops
    of query group tch-1, so the PE executes projection matmuls while the
    scalar engine runs exp and the DVE applies band masks. This keeps the
    PE dense (HAM stays un-throttled at 2.4 GHz) instead of stalling on the
    exp->mask->AV chain every head.
  - the output projection of query group g is likewise deferred and emitted
    as PE filler inside query group g+1's head slots.
  - AV matmuls and band-mask multiplies are trimmed to the in-band column
    span of each key block (the first key block of each head stays
    untrimmed so the whole yh PSUM bank is written by the start=True
    matmul before partial-span accumulations land on it).
  - softmax normalization: per head-pair, reciprocal_approx_fast (18-bit,
    5x faster than the iterative divide) of the sums rows, broadcast across
    64 partitions with a single K=2 matmul against a constant selector.
"""

import math

import numpy as np

import concourse.bass as bass
import concourse.mybir as mybir
import concourse.tile as tile
from concourse.bass_utils import run_bass_kernel_spmd

F32 = mybir.dt.float32
F32R = mybir.dt.float32r
BF16 = mybir.dt.bfloat16

B, T, C = 2, 2048, 1024
HS, DS = 8, 32
HL, DL = 8, 128
HD = 64
WIN_S, WIN_L = 256, 1024
NT = T // 128    # 16 t-blocks
NCB = C // 128   # 8 c-blocks
NG = T // 512    # 4 query groups
VW = HD + 1      # v columns + ones column for softmax sums
N_CORES = 8


def _split_waits(nc: bass.Bass) -> int:
    """Walrus in this env accepts at most 1 sync wait per instruction.
    Hoist extra waits onto same-engine InstNoOp instructions placed just
    before the owning instruction (same-engine program order preserves the
    blocking semantics)."""
    import bass_rust

    n_added = 0
    for f in nc.m.functions:
        for bb in f.blocks:
            insts = bb.instructions
            if not any(inst.sync_info and len(inst.sync_info.on_wait) > 1
                       for inst in insts):
                continue
            new = []
            for inst in insts:
                si = inst.sync_info
                waits = list(si.on_wait) if si else []
                if len(waits) > 1:
                    for i, w in enumerate(waits[:-1]):
                        nop = mybir.InstNoOp(
                            name=f"{inst.name}_hw{i}",
                            sync_info=bass_rust.SyncInfo(on_wait=[w], on_update=[]),
                            bass_nofuse=True,
                            engine=inst.engine,
                        )
                        new.append(nop)
                        n_added += 1
                    inst.sync_info = bass_rust.SyncInfo(
                        on_wait=waits[-1:], on_update=list(si.on_update))
                new.append(inst)
            bb.instructions = new
    return n_added


def _patch_tile_drain():
    """This walrus build rejects >1 sync wait on the TileContext tail drain
    ("Too many sync wait commands"). Re-emit the drain's waits as individual
    wait_ge instructions on the sync engine."""
    import bass_rust
    from concourse.tile import ScopedClock, TileContext

    def _drain_and_barrier(self, tick_clock, wait_clock):
        nc = self.nc
        drain_inst = nc.sync.drain()
        wait_clock.add_sem_waits(
            drain_inst.ins, ScopedClock({None: tick_clock.global_clock})
        )
        si = drain_inst.ins.sync_info
        waits = list(si.on_wait) if si is not None else []
        if len(waits) > 1:
            drain_inst.ins.sync_info = bass_rust.SyncInfo(on_wait=[], on_update=[])
            sems = {h.name: h for h in self.sems.allocated().values()}
            for w in waits:
                nc.sync.wait_ge(sems[w.ant_name], w.wait_value)
        nc.all_engine_barrier()
        popped = nc._tile_sem_poison_stack.pop()
        assert popped is self._sem_poison
        nc.clear_and_free_semaphores(list(self.sems.allocated().values()))
        nc.all_engine_barrier()

    TileContext._drain_and_barrier = _drain_and_barrier


_patch_tile_drain()


def _build_program() -> bass.Bass:
    nc = bass.Bass()

    xt_d = nc.dram_tensor("xt", [C, T], BF16, kind="ExternalInput")
    wsqk_d = nc.dram_tensor("wsqk", [C, 128], BF16, kind="ExternalInput")
    wql_d = nc.dram_tensor("wql", [C, 256], BF16, kind="ExternalInput")
    wkl_d = nc.dram_tensor("wkl", [C, 256], BF16, kind="ExternalInput")
    wv_d = nc.dram_tensor("wv", [C, 256], BF16, kind="ExternalInput")
    wp_d = nc.dram_tensor("wp", [256, C], BF16, kind="ExternalInput")
    bs_d = nc.dram_tensor("band_s", [128, WIN_S + 896], BF16, kind="ExternalInput")
    bl_d = nc.dram_tensor("band_l", [128, WIN_L + 896], BF16, kind="ExternalInput")
    ones_d = nc.dram_tensor("ones", [128, 64], BF16, kind="ExternalInput")
    out_d = nc.dram_tensor("out", [T, C], BF16, kind="ExternalOutput")

    scale_s = 1.0 / math.sqrt(DS)
    scale_l = 1.0 / math.sqrt(DL)

    with tile.TileContext(nc) as tc:
        with (
            tc.tile_pool(name="const", bufs=1) as const,
            tc.tile_pool(name="qkp", bufs=1) as qkp,
            tc.tile_pool(name="vp", bufs=1) as vp,
            tc.tile_pool(name="xtp", bufs=1) as xtp,
            tc.tile_pool(name="ptp", bufs=4) as ptp,
            tc.tile_pool(name="ytp", bufs=2) as ytp,
            tc.tile_pool(name="rbsp", bufs=2) as rbsp,
            tc.tile_pool(name="smallp", bufs=2) as smallp,
            tc.tile_pool(name="obp", bufs=3) as obp,
            tc.tile_pool(name="bigps", bufs=2, space="PSUM") as bigps,
            tc.tile_pool(name="p1", bufs=2, space="PSUM") as p1,
        ):
            # ---- DMA order: first projection chunk's dependencies first, so
            # the first matmul starts after ~2.5MB, not ~9MB.
            # per-cb weight DMAs spread across queues (a single 0.5MB DMA
            # serializes ~11us on one queue)
            wsqk = const.tile([128, NCB, 128], BF16, tag="wsqk", name="wsqk")
            for cb in range(NCB):
                nc.sync.dma_start(wsqk[:, cb, :], wsqk_d[cb * 128:(cb + 1) * 128, :])
            xt = [xtp.tile([128, T], BF16, tag=f"xt{cb}", name=f"xt{cb}")
                  for cb in range(NCB)]
            for cb in range(NCB):
                nc.sync.dma_start(
                    xt[cb][:, 0:512], xt_d[cb * 128:(cb + 1) * 128, 0:512])
            wql = const.tile([128, NCB, 256], BF16, tag="wql", name="wql")
            wkl = const.tile([128, NCB, 256], BF16, tag="wkl", name="wkl")
            wv = const.tile([128, NCB, 256], BF16, tag="wv", name="wv")
            for w_t, w_d in ((wql, wql_d), (wkl, wkl_d), (wv, wv_d)):
                for cb in range(NCB):
                    nc.sync.dma_start(w_t[:, cb, :], w_d[cb * 128:(cb + 1) * 128, :])
            wp0 = const.tile([128, C], BF16, tag="wp0", name="wp0")
            nc.sync.dma_start(wp0[:], wp_d[0:128, :])
            wp1 = const.tile([128, C], BF16, tag="wp1", name="wp1")
            nc.sync.dma_start(wp1[:], wp_d[128:256, :])
            band_s = const.tile([128, WIN_S + 896], BF16, tag="band_s", name="band_s")
            nc.sync.dma_start(band_s[:], bs_d[:, :])
            band_l = const.tile([128, WIN_L + 896], BF16, tag="band_l", name="band_l")
            nc.sync.dma_start(band_l[:], bl_d[:, :])
            onesb = const.tile([128, 64], BF16, tag="onesb", name="onesb")
            nc.sync.dma_start(onesb[:], ones_d[:, :])

            # ---- projection outputs (persist through the whole kernel) ----
            # q/k in bf16: walrus rejects mixed f32r/bf16 matmuls, and a bf16
            # moving operand runs 1 cyc/row at any N, so in-band span
            # trimming of the score matmuls pays.
            qts = qkp.tile([64, T], BF16, tag="qts", name="qts")
            kts = qkp.tile([64, T], BF16, tag="kts", name="kts")
            qtl = [qkp.tile([128, T], BF16, tag=f"qtl{h}", name=f"qtl{h}") for h in range(2)]
            ktl = [qkp.tile([128, T], BF16, tag=f"ktl{h}", name=f"ktl{h}") for h in range(2)]
            # v for all 4 heads in one tile: layout [128, (tb, head, vw)]
            vt = vp.tile([128, NT, 4, VW], BF16, tag="vt", name="vt")
            # ones column of each v block
            nc.sync.dma_start(vt[:, :, :, HD], ones_d[:, 0:4 * NT].rearrange("p (tb i) -> p tb i", i=4))

            # ---- rest of xT, chunk-ordered loads ----
            for tch in range(1, T // 512):
                for cb in range(NCB):
                    csl = (slice(None), slice(tch * 512, (tch + 1) * 512))
                    nc.sync.dma_start(
                        xt[cb][csl],
                        xt_d[cb * 128:(cb + 1) * 128, tch * 512:(tch + 1) * 512])

            # ================= emission helpers =================

            def make_proj_jobs(tch):
                """(qk_jobs, v_jobs): closures, each one PE accumulation job
                + PSUM drain."""
                tsl = slice(tch * 512, (tch + 1) * 512)
                jobs = []

                def qk_job(h, dsts, jidx):
                    def go():
                        ps = bigps.tile([128, 1024], F32, tag="bigps", name="bigps")
                        w = wsqk if h is None else (wql if dsts[0][2] == 'q' else wkl)
                        for cb in range(NCB):
                            lhsT = w[:, cb, :] if h is None else w[:, cb, h * 128:(h + 1) * 128]
                            nc.tensor.matmul(
                                ps[:, 0:512], lhsT, xt[cb][:, tsl],
                                start=(cb == 0), stop=(cb == NCB - 1))
                        with nc.allow_low_precision(reason="bf16 q"):
                            if h is None:
                                nc.vector.tensor_copy(qts[:, tsl], ps[0:64, 0:512])
                                nc.vector.tensor_copy(kts[:, tsl], ps[64:128, 0:512])
                            else:
                                dst = dsts[0][0]
                                nc.vector.tensor_copy(dst[:, tsl], ps[:, 0:512])
                    return go

                jobs.append(qk_job(None, [(None, None, 's')], 0))
                jobs.append(qk_job(0, [(qtl[0], None, 'q')], 1))
                jobs.append(qk_job(0, [(ktl[0], None, 'k')], 2))
                jobs.append(qk_job(1, [(qtl[1], None, 'q')], 3))
                jobs.append(qk_job(1, [(ktl[1], None, 'k')], 4))

                def v_job(tb):
                    def go():
                        ps = bigps.tile([128, 1024], F32, tag="bigps", name="bigps")
                        for cb in range(NCB):
                            nc.tensor.matmul(
                                ps[:, 0:256], xt[cb][:, tb * 128:(tb + 1) * 128], wv[:, cb, :],
                                start=(cb == 0), stop=(cb == NCB - 1))
                        with nc.allow_low_precision(reason="bf16 v"):
                            nc.vector.tensor_copy(
                                vt[:, tb, :, 0:HD],
                                ps[:, 0:256].rearrange("p (i d) -> p i d", d=HD))
                    return go

                vjobs = [v_job(tb) for tb in range(4 * tch, 4 * tch + 4)]
                return jobs, vjobs

            def head_params(qg, hi):
                q0 = qg * 512
                if hi < 2:
                    h = hi
                    return dict(
                        kt_ap=lambda kb, h=h: kts[32 * h: 32 * h + 32, kb * 128:(kb + 1) * 128],
                        qt_ap=qts[32 * h: 32 * h + 32, q0: q0 + 512],
                        win=WIN_S, scale=scale_s, band=band_s,
                    )
                h = hi - 2
                return dict(
                    kt_ap=lambda kb, h=h: ktl[h][:, kb * 128:(kb + 1) * 128],
                    qt_ap=qtl[h][:, q0: q0 + 512],
                    win=WIN_L, scale=scale_l, band=band_l,
                )

            def emit_scores(qg, head):
                """Trimmed score matmuls + exp + trimmed band masks.
                Returns [(kb, pt, jj, a, b)] for the AV stage."""
                p = head_params(qg, head)
                q0 = qg * 512
                win = p['win']
                kb_lo = max(0, q0 - win) // 128
                kb_hi = (q0 + 384) // 128
                kbs = list(range(kb_lo, kb_hi + 1))
                # in-band column span per key block; first kb untrimmed so
                # the start=True AV matmul covers the whole yh bank
                spans = []
                for kb in kbs:
                    delta = kb * 128 - q0
                    first = (kb == kbs[0])
                    a = 0 if first else max(0, delta)
                    b = 512 if first else min(512, delta + win + 128)
                    spans.append((a, b))
                out = []
                for j in range(0, len(kbs), 2):
                    pair = kbs[j: j + 2]
                    st = bigps.tile([128, 1024], F32, tag="bigps", name="bigps")
                    for jj, kb in enumerate(pair):
                        a, b = spans[j + jj]
                        nc.tensor.matmul(
                            st[:, jj * 512 + a: jj * 512 + b],
                            p['kt_ap'](kb), p['qt_ap'][:, a:b],
                            start=True, stop=True)
                    # one exp over the pair's span superset (unwritten gap
                    # columns produce garbage pt values nobody reads)
                    ea = spans[j][0]
                    eb = 512 * (len(pair) - 1) + spans[j + len(pair) - 1][1]
                    pt = ptp.tile([128, 1024], BF16, tag="pt", name="pt")
                    with nc.allow_low_precision(reason="bf16 softmax probs"):
                        nc.scalar.activation(
                            pt[:, ea:eb], st[:, ea:eb],
                            mybir.ActivationFunctionType.Exp, scale=p['scale'])
                    for jj, kb in enumerate(pair):
                        delta = kb * 128 - q0
                        a, b = spans[j + jj]
                        masked = not (512 - win <= delta <= -128)
                        if masked:
                            off = 384 - delta
                            eng = nc.vector if head < 2 else nc.gpsimd
                            psl = (slice(None), slice(jj * 512 + a, jj * 512 + b))
                            with nc.allow_low_precision(reason="bf16 mask"):
                                eng.tensor_tensor(
                                    out=pt[psl], in0=pt[psl],
                                    in1=p['band'][:, off + a: off + b],
                                    op=mybir.AluOpType.mult)
                        out.append((kb, pt, jj, a, b))
                return out

            def emit_av(qg, head, slices, sp, row):
                yh = p1.tile([128, 512], F32, tag="yh", name="yh")
                for i, (kb, pt, jj, a, b) in enumerate(slices):
                    nc.tensor.matmul(
                        yh[0:VW, a:b], vt[:, kb, head, :],
                        pt[:, jj * 512 + a: jj * 512 + b],
                        start=(i == 0), stop=(i == len(slices) - 1))
                # stage the softmax sums row into the pair tile (rows 0/32)
                nc.vector.tensor_copy(sp[row:row + 1, :], yh[HD: HD + 1, :])
                return yh

            def emit_norm(yts_pair, yh_pair, sp):
                # 1/sums as exp(-ln(sums)) on the scalar engine, batched over
                # the head pair (rows 0 and 32; DVE/ACT time only depends on
                # the free size). Both funcs live in one ACT table set.
                lp = smallp.tile([33, 512], F32, tag="ll", name="ll")
                nc.scalar.activation(lp[:, :], sp[:, :],
                                     mybir.ActivationFunctionType.Ln)
                rp = smallp.tile([33, 512], BF16, tag="rr", name="rr")
                with nc.allow_low_precision(reason="bf16 softmax recip"):
                    nc.scalar.activation(rp[:, :], lp[:, :],
                                         mybir.ActivationFunctionType.Exp, scale=-1.0)
                rbs_t = rbsp.tile([128, 512], F32, tag="rbs", name="rbs")
                for k in (0, 1):
                    rb = p1.tile([128, 512], F32, tag="pr", name="pr")
                    nc.tensor.matmul(rb[0:64, :], onesb[32 * k: 32 * k + 1, 0:64],
                                     rp[32 * k: 32 * k + 1, :], start=True, stop=True)
                    nc.vector.tensor_copy(rbs_t[64 * k: 64 * k + 64, :], rb[0:64, :])
                with nc.allow_low_precision(reason="bf16 attn out"):
                    for k in (0, 1):
                        nc.vector.tensor_mul(
                            yts_pair[64 * k: 64 * k + 64, :],
                            yh_pair[k][0:HD, :], rbs_t[64 * k: 64 * k + 64, :])

            def emit_outproj_sub(qg, sub, yts_qg):
                """One 128-query sub-block of query group qg's out-projection."""
                qs = qg * 512 + sub * 128
                ssl = (slice(None), slice(sub * 128, (sub + 1) * 128))
                ob = obp.tile([128, 1024], BF16, tag="ob", name="ob")
                with nc.allow_low_precision(reason="bf16 out"):
                    for nh in range(2):
                        po = p1.tile([128, 512], F32, tag="pr", name="pr")
                        nc.tensor.matmul(po[:, :], yts_qg[0][ssl], wp0[:, nh * 512:(nh + 1) * 512],
                                         start=True, stop=False)
                        nc.tensor.matmul(po[:, :], yts_qg[1][ssl], wp1[:, nh * 512:(nh + 1) * 512],
                                         start=False, stop=True)
                        if nh == 0:
                            nc.vector.tensor_copy(ob[:, 0:512], po[:, :])
                        else:
                            nc.scalar.copy(ob[:, 512:1024], po[:, :])
                nc.sync.dma_start(out_d[qs: qs + 128, :], ob[:])

            # ================= main schedule =================

            # prologue: projection chunk 0
            jq0, jv0 = make_proj_jobs(0)
            for job in jq0 + jv0:
                job()

            # heads in order [long0, long1, short0, short1]: the long heads'
            # deeper exp chains land early where PE filler is richest, and
            # the light short heads form the pipeline tail.
            HEAD_ORDER = [2, 3, 0, 1]
            prev_yts = None
            deferred_v = []
            for qg in range(NG):
                if qg + 1 < NG:
                    jq, jv = make_proj_jobs(qg + 1)
                    if qg + 1 == NG - 1:
                        # tch3's v blocks are only needed by qg3's AV stage:
                        # keep them as qg3's slot-0 PE filler
                        pjobs, deferred_v = jq, jv
                    else:
                        pjobs = jq + jv
                else:
                    pjobs = deferred_v
                takes = [2, 2, 2, 3] if qg < 2 else ([2, 2, 1, 0] if qg == 2 else [4, 0, 0, 0])
                pj = 0
                yts = [ytp.tile([128, 512], BF16, tag=f"yts{i}", name=f"yts{i}")
                       for i in range(2)]
                yhs = {}
                sp = None
                for slot, head in enumerate(HEAD_ORDER):
                    if slot % 2 == 0:
                        sp = smallp.tile([33, 512], F32, tag="sp", name="sp")
                        nc.gpsimd.memset(sp[:, :], 1.0)
                    slices = emit_scores(qg, head)
                    # PE filler while exp/mask run on scalar/DVE:
                    if prev_yts is not None:
                        emit_outproj_sub(qg - 1, slot, prev_yts)
                    for _ in range(takes[slot]):
                        if pj < len(pjobs):
                            pjobs[pj]()
                            pj += 1
                    yhs[head] = emit_av(qg, head, slices, sp, row=32 * (slot % 2))
                    if slot == 1:
                        emit_norm(yts[1], [yhs[2], yhs[3]], sp)
                    elif slot == 3:
                        emit_norm(yts[0], [yhs[0], yhs[1]], sp)
                while pj < len(pjobs):
                    pjobs[pj]()
                    pj += 1
                prev_yts = yts

            # epilogue: last query group's out-projection
            for sub in range(4):
                emit_outproj_sub(NG - 1, sub, prev_yts)

    return nc


_PROGRAM = None


def _get_program() -> bass.Bass:
    global _PROGRAM
    if _PROGRAM is None:
        _PROGRAM = _build_program()
        _split_waits(_PROGRAM)
    return _PROGRAM


def _band_image(win: int) -> np.ndarray:
    """[128, win+896] 0/1 image: B[r, u] = 1 iff (u - 384 - r) in [0, win)."""
    u = np.arange(win + 896)[None, :]
    r = np.arange(128)[:, None]
    d = u - 384 - r
    return ((d >= 0) & (d < win)).astype(np.float32)


def make_in_maps(x, Wqk_short, Wv_short, Wqk_long, Wv_long, Wproj):
    """Host-side sharding: per-core input dict for core c = 4*b + g."""
    import ml_dtypes

    bf16 = ml_dtypes.bfloat16
    x = np.asarray(x, dtype=np.float32)
    Wqk_short = np.asarray(Wqk_short, dtype=np.float32).astype(bf16)
    Wv_short = np.asarray(Wv_short, dtype=np.float32).astype(bf16)
    Wqk_long = np.asarray(Wqk_long, dtype=np.float32).astype(bf16)
    Wv_long = np.asarray(Wv_long, dtype=np.float32).astype(bf16)
    Wproj = np.asarray(Wproj, dtype=np.float32)
    assert x.shape == (B, T, C)

    xts = [np.ascontiguousarray(x[b].T.astype(bf16)) for b in range(B)]
    band_s = _band_image(WIN_S).astype(bf16)
    band_l = _band_image(WIN_L).astype(bf16)
    ones = np.ones((128, 64), dtype=bf16)
    in_maps = []
    for c in range(N_CORES):
        b, g = divmod(c, 4)
        wsqk = np.ascontiguousarray(np.concatenate(
            [Wqk_short[:, g * 64:(g + 1) * 64],
             Wqk_short[:, 256 + g * 64: 256 + (g + 1) * 64]], axis=1))
        wql = np.ascontiguousarray(Wqk_long[:, g * 256:(g + 1) * 256])
        wkl = np.ascontiguousarray(Wqk_long[:, 1024 + g * 256: 1024 + (g + 1) * 256])
        wv = np.ascontiguousarray(np.concatenate(
            [Wv_short[:, g * 128:(g + 1) * 128],
             Wv_long[:, g * 128:(g + 1) * 128]], axis=1))
        wp = np.ascontiguousarray(np.concatenate(
            [Wproj[g * 128:(g + 1) * 128, :],
             Wproj[512 + g * 128: 512 + (g + 1) * 128, :]], axis=0).astype(bf16))
        in_maps.append({
            "xt": xts[b], "wsqk": wsqk, "wql": wql, "wkl": wkl, "wv": wv, "wp": wp,
            "band_s": band_s, "band_l": band_l, "ones": ones,
        })
    return in_maps


def gather(results) -> np.ndarray:
    out = np.empty((B, T, C), dtype=np.float32)
    for b in range(B):
        acc = np.zeros((T, C), dtype=np.float64)
        for g in range(4):
            acc += np.asarray(results[4 * b + g]["out"], dtype=np.float32)
        out[b] = acc.astype(np.float32)
    return out


def kernel(x, Wqk_short, Wv_short, Wqk_long, Wv_long, Wproj, **run_kwargs):
    nc = _get_program()
    in_maps = make_in_maps(x, Wqk_short, Wv_short, Wqk_long, Wv_long, Wproj)
    res = run_bass_kernel_spmd(nc, in_maps, core_ids=list(range(N_CORES)), **run_kwargs)
    out = gather(res.results)
    if run_kwargs:
        kernel.last_results = res
    return out


# revision 35
# speedup vs baseline: 1.0053x; 1.0053x over previous
"""Trainium2 Bass kernel for a two-window sparse causal self-attention block.

Model (B=2, T=2048, C=1024):
  - 8 "short" heads: d_qk=32,  window 256
  - 8 "long"  heads: d_qk=128, window 1024
  - value/output head dim 64, output projection C x C.

Sharding (8 cores): data-parallel over batch (2) x head-parallel over head
groups (4). Core c = 4*b + g handles batch b and heads {2g, 2g+1} of both the
short and long sets. Each core computes its 4 heads' attention plus the
corresponding 256 rows of Wproj, producing a partial [T, C] output; the host
sums the 4 partials per batch element.

Device-side design notes (v2, software-pipelined):
  - scores in f32r (full PE rate at N=512, exact fp32 bits); p/v/Wproj/y in
    bf16 (any-N full rate, 2x DVE modes). Softmax sums kept exact in fp32
    PSUM via a bf16 ones column appended to v.
  - everything computed transposed so no on-device transposes: host passes
    xT [C, T]; projections give qT/kT [d, T] and v [T, dv]; sT[k, q] =
    kT.T @ qT; yT[dv, q] = v_aug.T @ pT.
  - the projection work is software-pipelined into the attention loop:
    projection chunk tch (512 tokens) is emitted between the attention ops
    of query group tch-1, so the PE executes projection matmuls while the
    scalar engine runs exp and the DVE applies band masks. This keeps the
    PE dense (HAM stays un-throttled at 2.4 GHz) instead of stalling on the
    exp->mask->AV chain every head.
  - the output projection of query group g is likewise deferred and emitted
    as PE filler inside query group g+1's head slots.
  - AV matmuls and band-mask multiplies are trimmed to the in-band column
    span of each key block (the first key block of each head stays
    untrimmed so the whole yh PSUM bank is written by the start=True
    matmul before partial-span accumulations land on it).
  - softmax normalization: per head-pair, reciprocal_approx_fast (18-bit,
    5x faster than the iterative divide) of the sums rows, broadcast across
    64 partitions with a single K=2 matmul against a constant selector.
"""

import math

import numpy as np

import concourse.bass as bass
import concourse.mybir as mybir
import concourse.tile as tile
from concourse.bass_utils import run_bass_kernel_spmd

F32 = mybir.dt.float32
F32R = mybir.dt.float32r
BF16 = mybir.dt.bfloat16

B, T, C = 2, 2048, 1024
HS, DS = 8, 32
HL, DL = 8, 128
HD = 64
WIN_S, WIN_L = 256, 1024
NT = T // 128    # 16 t-blocks
NCB = C // 128   # 8 c-blocks
NG = T // 512    # 4 query groups
VW = HD + 1      # v columns + ones column for softmax sums
N_CORES = 8


def _split_waits(nc: bass.Bass) -> int:
    """Walrus in this env accepts at most 1 sync wait per instruction.
    Hoist extra waits onto same-engine InstNoOp instructions placed just
    before the owning instruction (same-engine program order preserves the
    blocking semantics)."""
    import bass_rust

    n_added = 0
    for f in nc.m.functions:
        for bb in f.blocks:
            insts = bb.instructions
            if not any(inst.sync_info and len(inst.sync_info.on_wait) > 1
                       for inst in insts):
                continue
            new = []
            for inst in insts:
                si = inst.sync_info
                waits = list(si.on_wait) if si else []
                if len(waits) > 1:
                    for i, w in enumerate(waits[:-1]):
                        nop = mybir.InstNoOp(
                            name=f"{inst.name}_hw{i}",
                            sync_info=bass_rust.SyncInfo(on_wait=[w], on_update=[]),
                            bass_nofuse=True,
                            engine=inst.engine,
                        )
                        new.append(nop)
                        n_added += 1
                    inst.sync_info = bass_rust.SyncInfo(
                        on_wait=waits[-1:], on_update=list(si.on_update))
                new.append(inst)
            bb.instructions = new
    return n_added


def _patch_tile_drain():
    """This walrus build rejects >1 sync wait on the TileContext tail drain
    ("Too many sync wait commands"). Re-emit the drain's waits as individual
    wait_ge instructions on the sync engine."""
    import bass_rust
    from concourse.tile import ScopedClock, TileContext

    def _drain_and_barrier(self, tick_clock, wait_clock):
        nc = self.nc
        drain_inst = nc.sync.drain()
        wait_clock.add_sem_waits(
            drain_inst.ins, ScopedClock({None: tick_clock.global_clock})
        )
        si = drain_inst.ins.sync_info
        waits = list(si.on_wait) if si is not None else []
        if len(waits) > 1:
            drain_inst.ins.sync_info = bass_rust.SyncInfo(on_wait=[], on_update=[])
            sems = {h.name: h for h in self.sems.allocated().values()}
            for w in waits:
                nc.sync.wait_ge(sems[w.ant_name], w.wait_value)
        nc.all_engine_barrier()
        popped = nc._tile_sem_poison_stack.pop()
        assert popped is self._sem_poison
        nc.clear_and_free_semaphores(list(self.sems.allocated().values()))
        nc.all_engine_barrier()

    TileContext._drain_and_barrier = _drain_and_barrier


_patch_tile_drain()


def _build_program() -> bass.Bass:
    nc = bass.Bass()

    xt_d = nc.dram_tensor("xt", [C, T], BF16, kind="ExternalInput")
    wsqk_d = nc.dram_tensor("wsqk", [C, 128], BF16, kind="ExternalInput")
    wql_d = nc.dram_tensor("wql", [C, 256], BF16, kind="ExternalInput")
    wkl_d = nc.dram_tensor("wkl", [C, 256], BF16, kind="ExternalInput")
    wv_d = nc.dram_tensor("wv", [C, 256], BF16, kind="ExternalInput")
    wp_d = nc.dram_tensor("wp", [256, C], BF16, kind="ExternalInput")
    bs_d = nc.dram_tensor("band_s", [128, WIN_S + 896], BF16, kind="ExternalInput")
    bl_d = nc.dram_tensor("band_l", [128, WIN_L + 896], BF16, kind="ExternalInput")
    ones_d = nc.dram_tensor("ones", [128, 64], BF16, kind="ExternalInput")
    out_d = nc.dram_tensor("out", [T, C], BF16, kind="ExternalOutput")

    scale_s = 1.0 / math.sqrt(DS)
    scale_l = 1.0 / math.sqrt(DL)

    with tile.TileContext(nc) as tc:
        with (
            tc.tile_pool(name="const", bufs=1) as const,
            tc.tile_pool(name="qkp", bufs=1) as qkp,
            tc.tile_pool(name="vp", bufs=1) as vp,
            tc.tile_pool(name="xtp", bufs=1) as xtp,
            tc.tile_pool(name="ptp", bufs=4) as ptp,
            tc.tile_pool(name="ytp", bufs=2) as ytp,
            tc.tile_pool(name="rbsp", bufs=2) as rbsp,
            tc.tile_pool(name="smallp", bufs=2) as smallp,
            tc.tile_pool(name="obp", bufs=3) as obp,
            tc.tile_pool(name="bigps", bufs=2, space="PSUM") as bigps,
            tc.tile_pool(name="p1", bufs=2, space="PSUM") as p1,
        ):
            # ---- DMA order: first projection chunk's dependencies first, so
            # the first matmul starts after ~2.5MB, not ~9MB.
            # per-cb weight DMAs spread across queues (a single 0.5MB DMA
            # serializes ~11us on one queue)
            wsqk = const.tile([128, NCB, 128], BF16, tag="wsqk", name="wsqk")
            for cb in range(NCB):
                nc.sync.dma_start(wsqk[:, cb, :], wsqk_d[cb * 128:(cb + 1) * 128, :])
            xt = [xtp.tile([128, T], BF16, tag=f"xt{cb}", name=f"xt{cb}")
                  for cb in range(NCB)]
            for cb in range(NCB):
                nc.sync.dma_start(
                    xt[cb][:, 0:512], xt_d[cb * 128:(cb + 1) * 128, 0:512])
            wql = const.tile([128, NCB, 256], BF16, tag="wql", name="wql")
            wkl = const.tile([128, NCB, 256], BF16, tag="wkl", name="wkl")
            wv = const.tile([128, NCB, 256], BF16, tag="wv", name="wv")
            for w_t, w_d in ((wql, wql_d), (wkl, wkl_d), (wv, wv_d)):
                for cb in range(NCB):
                    nc.sync.dma_start(w_t[:, cb, :], w_d[cb * 128:(cb + 1) * 128, :])
            # bands before the bulk x chunks: qg0's masks need them early
            band_s = const.tile([128, WIN_S + 896], BF16, tag="band_s", name="band_s")
            nc.sync.dma_start(band_s[:], bs_d[:, :])
            band_l = const.tile([128, WIN_L + 896], BF16, tag="band_l", name="band_l")
            nc.sync.dma_start(band_l[:], bl_d[:, :])
            onesb = const.tile([128, 64], BF16, tag="onesb", name="onesb")
            nc.sync.dma_start(onesb[:], ones_d[:, :])
            # x chunk 1 next: qg0's interleaved projection jobs consume it
            for cb in range(NCB):
                nc.sync.dma_start(
                    xt[cb][:, 512:1024], xt_d[cb * 128:(cb + 1) * 128, 512:1024])
            wp0 = const.tile([128, C], BF16, tag="wp0", name="wp0")
            nc.sync.dma_start(wp0[:], wp_d[0:128, :])
            wp1 = const.tile([128, C], BF16, tag="wp1", name="wp1")
            nc.sync.dma_start(wp1[:], wp_d[128:256, :])

            # ---- projection outputs (persist through the whole kernel) ----
            # q/k in bf16: walrus rejects mixed f32r/bf16 matmuls, and a bf16
            # moving operand runs 1 cyc/row at any N, so in-band span
            # trimming of the score matmuls pays.
            qts = qkp.tile([64, T], BF16, tag="qts", name="qts")
            kts = qkp.tile([64, T], BF16, tag="kts", name="kts")
            qtl = [qkp.tile([128, T], BF16, tag=f"qtl{h}", name=f"qtl{h}") for h in range(2)]
            ktl = [qkp.tile([128, T], BF16, tag=f"ktl{h}", name=f"ktl{h}") for h in range(2)]
            # v for all 4 heads in one tile: layout [128, (tb, head, vw)]
            vt = vp.tile([128, NT, 4, VW], BF16, tag="vt", name="vt")
            # ones column of each v block
            nc.sync.dma_start(vt[:, :, :, HD], ones_d[:, 0:4 * NT].rearrange("p (tb i) -> p tb i", i=4))

            # ---- rest of xT, chunk-ordered loads ----
            for tch in range(2, T // 512):
                for cb in range(NCB):
                    csl = (slice(None), slice(tch * 512, (tch + 1) * 512))
                    nc.sync.dma_start(
                        xt[cb][csl],
                        xt_d[cb * 128:(cb + 1) * 128, tch * 512:(tch + 1) * 512])

            # ================= emission helpers =================

            def make_proj_jobs(tch):
                """(qk_jobs, v_jobs): closures, each one PE accumulation job
                + PSUM drain."""
                tsl = slice(tch * 512, (tch + 1) * 512)
                jobs = []

                def qk_job(h, dsts, jidx):
                    def go():
                        ps = bigps.tile([128, 1024], F32, tag="bigps", name="bigps")
                        w = wsqk if h is None else (wql if dsts[0][2] == 'q' else wkl)
                        for cb in range(NCB):
                            lhsT = w[:, cb, :] if h is None else w[:, cb, h * 128:(h + 1) * 128]
                            nc.tensor.matmul(
                                ps[:, 0:512], lhsT, xt[cb][:, tsl],
                                start=(cb == 0), stop=(cb == NCB - 1))
                        with nc.allow_low_precision(reason="bf16 q"):
                            if h is None:
                                nc.vector.tensor_copy(qts[:, tsl], ps[0:64, 0:512])
                                nc.vector.tensor_copy(kts[:, tsl], ps[64:128, 0:512])
                            else:
                                dst = dsts[0][0]
                                nc.vector.tensor_copy(dst[:, tsl], ps[:, 0:512])
                    return go

                jobs.append(qk_job(None, [(None, None, 's')], 0))
                jobs.append(qk_job(0, [(qtl[0], None, 'q')], 1))
                jobs.append(qk_job(0, [(ktl[0], None, 'k')], 2))
                jobs.append(qk_job(1, [(qtl[1], None, 'q')], 3))
                jobs.append(qk_job(1, [(ktl[1], None, 'k')], 4))

                def v_job(tb):
                    def go():
                        ps = bigps.tile([128, 1024], F32, tag="bigps", name="bigps")
                        for cb in range(NCB):
                            nc.tensor.matmul(
                                ps[:, 0:256], xt[cb][:, tb * 128:(tb + 1) * 128], wv[:, cb, :],
                                start=(cb == 0), stop=(cb == NCB - 1))
                        with nc.allow_low_precision(reason="bf16 v"):
                            nc.vector.tensor_copy(
                                vt[:, tb, :, 0:HD],
                                ps[:, 0:256].rearrange("p (i d) -> p i d", d=HD))
                    return go

                vjobs = [v_job(tb) for tb in range(4 * tch, 4 * tch + 4)]
                return jobs, vjobs

            def head_params(qg, hi):
                q0 = qg * 512
                if hi < 2:
                    h = hi
                    return dict(
                        kt_ap=lambda kb, h=h: kts[32 * h: 32 * h + 32, kb * 128:(kb + 1) * 128],
                        qt_ap=qts[32 * h: 32 * h + 32, q0: q0 + 512],
                        win=WIN_S, scale=scale_s, band=band_s,
                    )
                h = hi - 2
                return dict(
                    kt_ap=lambda kb, h=h: ktl[h][:, kb * 128:(kb + 1) * 128],
                    qt_ap=qtl[h][:, q0: q0 + 512],
                    win=WIN_L, scale=scale_l, band=band_l,
                )

            def emit_scores(qg, head):
                """Trimmed score matmuls + exp + trimmed band masks.
                Returns [(kb, pt, jj, a, b)] for the AV stage."""
                p = head_params(qg, head)
                q0 = qg * 512
                win = p['win']
                kb_lo = max(0, q0 - win) // 128
                kb_hi = (q0 + 384) // 128
                kbs = list(range(kb_lo, kb_hi + 1))
                # in-band column span per key block; first kb untrimmed so
                # the start=True AV matmul covers the whole yh bank
                spans = []
                for kb in kbs:
                    delta = kb * 128 - q0
                    first = (kb == kbs[0])
                    a = 0 if first else max(0, delta)
                    b = 512 if first else min(512, delta + win + 128)
                    spans.append((a, b))
                out = []
                for j in range(0, len(kbs), 2):
                    pair = kbs[j: j + 2]
                    st = bigps.tile([128, 1024], F32, tag="bigps", name="bigps")
                    for jj, kb in enumerate(pair):
                        a, b = spans[j + jj]
                        nc.tensor.matmul(
                            st[:, jj * 512 + a: jj * 512 + b],
                            p['kt_ap'](kb), p['qt_ap'][:, a:b],
                            start=True, stop=True)
                    # one exp over the pair's span superset (unwritten gap
                    # columns produce garbage pt values nobody reads)
                    ea = spans[j][0]
                    eb = 512 * (len(pair) - 1) + spans[j + len(pair) - 1][1]
                    pt = ptp.tile([128, 1024], BF16, tag="pt", name="pt")
                    with nc.allow_low_precision(reason="bf16 softmax probs"):
                        nc.scalar.activation(
                            pt[:, ea:eb], st[:, ea:eb],
                            mybir.ActivationFunctionType.Exp, scale=p['scale'])
                    for jj, kb in enumerate(pair):
                        delta = kb * 128 - q0
                        a, b = spans[j + jj]
                        masked = not (512 - win <= delta <= -128)
                        if masked:
                            off = 384 - delta
                            eng = nc.vector if head < 2 else nc.gpsimd
                            psl = (slice(None), slice(jj * 512 + a, jj * 512 + b))
                            with nc.allow_low_precision(reason="bf16 mask"):
                                eng.tensor_tensor(
                                    out=pt[psl], in0=pt[psl],
                                    in1=p['band'][:, off + a: off + b],
                                    op=mybir.AluOpType.mult)
                        out.append((kb, pt, jj, a, b))
                return out

            def emit_av(qg, head, slices, sp, row):
                yh = p1.tile([128, 512], F32, tag="yh", name="yh")
                for i, (kb, pt, jj, a, b) in enumerate(slices):
                    nc.tensor.matmul(
                        yh[0:VW, a:b], vt[:, kb, head, :],
                        pt[:, jj * 512 + a: jj * 512 + b],
                        start=(i == 0), stop=(i == len(slices) - 1))
                # stage the softmax sums row into the pair tile (rows 0/32)
                nc.vector.tensor_copy(sp[row:row + 1, :], yh[HD: HD + 1, :])
                return yh

            def emit_norm(yts_pair, yh_pair, sp):
                # 1/sums as exp(-ln(sums)) on the scalar engine, batched over
                # the head pair (rows 0 and 32; DVE/ACT time only depends on
                # the free size). Both funcs live in one ACT table set.
                lp = smallp.tile([33, 512], F32, tag="ll", name="ll")
                nc.scalar.activation(lp[:, :], sp[:, :],
                                     mybir.ActivationFunctionType.Ln)
                rp = smallp.tile([33, 512], BF16, tag="rr", name="rr")
                with nc.allow_low_precision(reason="bf16 softmax recip"):
                    nc.scalar.activation(rp[:, :], lp[:, :],
                                         mybir.ActivationFunctionType.Exp, scale=-1.0)
                rbs_t = rbsp.tile([128, 512], F32, tag="rbs", name="rbs")
                for k in (0, 1):
                    rb = p1.tile([128, 512], F32, tag="pr", name="pr")
                    nc.tensor.matmul(rb[0:64, :], onesb[32 * k: 32 * k + 1, 0:64],
                                     rp[32 * k: 32 * k + 1, :], start=True, stop=True)
                    nc.vector.tensor_copy(rbs_t[64 * k: 64 * k + 64, :], rb[0:64, :])
                with nc.allow_low_precision(reason="bf16 attn out"):
                    for k in (0, 1):
                        nc.vector.tensor_mul(
                            yts_pair[64 * k: 64 * k + 64, :],
                            yh_pair[k][0:HD, :], rbs_t[64 * k: 64 * k + 64, :])

            def emit_outproj_sub(qg, sub, yts_qg):
                """One 128-query sub-block of query group qg's out-projection."""
                qs = qg * 512 + sub * 128
                ssl = (slice(None), slice(sub * 128, (sub + 1) * 128))
                ob = obp.tile([128, 1024], BF16, tag="ob", name="ob")
                with nc.allow_low_precision(reason="bf16 out"):
                    for nh in range(2):
                        po = p1.tile([128, 512], F32, tag="pr", name="pr")
                        nc.tensor.matmul(po[:, :], yts_qg[0][ssl], wp0[:, nh * 512:(nh + 1) * 512],
                                         start=True, stop=False)
                        nc.tensor.matmul(po[:, :], yts_qg[1][ssl], wp1[:, nh * 512:(nh + 1) * 512],
                                         start=False, stop=True)
                        if nh == 0:
                            nc.vector.tensor_copy(ob[:, 0:512], po[:, :])
                        else:
                            nc.scalar.copy(ob[:, 512:1024], po[:, :])
                nc.sync.dma_start(out_d[qs: qs + 128, :], ob[:])

            # ================= main schedule =================

            # prologue: projection chunk 0
            jq0, jv0 = make_proj_jobs(0)
            for job in jq0 + jv0:
                job()

            # heads in order [long0, long1, short0, short1]: the long heads'
            # deeper exp chains land early where PE filler is richest, and
            # the light short heads form the pipeline tail.
            HEAD_ORDER = [2, 3, 0, 1]
            prev_yts = None
            deferred_v = []
            for qg in range(NG):
                if qg + 1 < NG:
                    jq, jv = make_proj_jobs(qg + 1)
                    if qg + 1 == NG - 1:
                        # tch3's v blocks are only needed by qg3's AV stage:
                        # keep them as qg3's slot-0 PE filler
                        pjobs, deferred_v = jq, jv
                    else:
                        pjobs = jq + jv
                else:
                    pjobs = deferred_v
                takes = [2, 2, 2, 3] if qg < 2 else ([2, 2, 1, 0] if qg == 2 else [4, 0, 0, 0])
                pj = 0
                yts = [ytp.tile([128, 512], BF16, tag=f"yts{i}", name=f"yts{i}")
                       for i in range(2)]
                yhs = {}
                sp = None
                for slot, head in enumerate(HEAD_ORDER):
                    if slot % 2 == 0:
                        sp = smallp.tile([33, 512], F32, tag="sp", name="sp")
                        nc.gpsimd.memset(sp[:, :], 1.0)
                    slices = emit_scores(qg, head)
                    # PE filler while exp/mask run on scalar/DVE:
                    if prev_yts is not None:
                        emit_outproj_sub(qg - 1, slot, prev_yts)
                    for _ in range(takes[slot]):
                        if pj < len(pjobs):
                            pjobs[pj]()
                            pj += 1
                    yhs[head] = emit_av(qg, head, slices, sp, row=32 * (slot % 2))
                    if slot == 1:
                        emit_norm(yts[1], [yhs[2], yhs[3]], sp)
                    elif slot == 3:
                        emit_norm(yts[0], [yhs[0], yhs[1]], sp)
                while pj < len(pjobs):
                    pjobs[pj]()
                    pj += 1
                prev_yts = yts

            # epilogue: last query group's out-projection
            for sub in range(4):
                emit_outproj_sub(NG - 1, sub, prev_yts)

    return nc


_PROGRAM = None


def _get_program() -> bass.Bass:
    global _PROGRAM
    if _PROGRAM is None:
        _PROGRAM = _build_program()
        _split_waits(_PROGRAM)
    return _PROGRAM


def _band_image(win: int) -> np.ndarray:
    """[128, win+896] 0/1 image: B[r, u] = 1 iff (u - 384 - r) in [0, win)."""
    u = np.arange(win + 896)[None, :]
    r = np.arange(128)[:, None]
    d = u - 384 - r
    return ((d >= 0) & (d < win)).astype(np.float32)


def make_in_maps(x, Wqk_short, Wv_short, Wqk_long, Wv_long, Wproj):
    """Host-side sharding: per-core input dict for core c = 4*b + g."""
    import ml_dtypes

    bf16 = ml_dtypes.bfloat16
    x = np.asarray(x, dtype=np.float32)
    Wqk_short = np.asarray(Wqk_short, dtype=np.float32).astype(bf16)
    Wv_short = np.asarray(Wv_short, dtype=np.float32).astype(bf16)
    Wqk_long = np.asarray(Wqk_long, dtype=np.float32).astype(bf16)
    Wv_long = np.asarray(Wv_long, dtype=np.float32).astype(bf16)
    Wproj = np.asarray(Wproj, dtype=np.float32)
    assert x.shape == (B, T, C)

    xts = [np.ascontiguousarray(x[b].T.astype(bf16)) for b in range(B)]
    band_s = _band_image(WIN_S).astype(bf16)
    band_l = _band_image(WIN_L).astype(bf16)
    ones = np.ones((128, 64), dtype=bf16)
    in_maps = []
    for c in range(N_CORES):
        b, g = divmod(c, 4)
        wsqk = np.ascontiguousarray(np.concatenate(
            [Wqk_short[:, g * 64:(g + 1) * 64],
             Wqk_short[:, 256 + g * 64: 256 + (g + 1) * 64]], axis=1))
        wql = np.ascontiguousarray(Wqk_long[:, g * 256:(g + 1) * 256])
        wkl = np.ascontiguousarray(Wqk_long[:, 1024 + g * 256: 1024 + (g + 1) * 256])
        wv = np.ascontiguousarray(np.concatenate(
            [Wv_short[:, g * 128:(g + 1) * 128],
             Wv_long[:, g * 128:(g + 1) * 128]], axis=1))
        wp = np.ascontiguousarray(np.concatenate(
            [Wproj[g * 128:(g + 1) * 128, :],
             Wproj[512 + g * 128: 512 + (g + 1) * 128, :]], axis=0).astype(bf16))
        in_maps.append({
            "xt": xts[b], "wsqk": wsqk, "wql": wql, "wkl": wkl, "wv": wv, "wp": wp,
            "band_s": band_s, "band_l": band_l, "ones": ones,
        })
    return in_maps


def gather(results) -> np.ndarray:
    out = np.empty((B, T, C), dtype=np.float32)
    for b in range(B):
        acc = np.zeros((T, C), dtype=np.float64)
        for g in range(4):
            acc += np.asarray(results[4 * b + g]["out"], dtype=np.float32)
        out[b] = acc.astype(np.float32)
    return out


def kernel(x, Wqk_short, Wv_short, Wqk_long, Wv_long, Wproj, **run_kwargs):
    nc = _get_program()
    in_maps = make_in_maps(x, Wqk_short, Wv_short, Wqk_long, Wv_long, Wproj)
    res = run_bass_kernel_spmd(nc, in_maps, core_ids=list(range(N_CORES)), **run_kwargs)
    out = gather(res.results)
    if run_kwargs:
        kernel.last_results = res
    return out


# revision 38
# speedup vs baseline: 1.1004x; 1.0946x over previous
"""Trainium2 Bass kernel for a two-window sparse causal self-attention block.

Model (B=2, T=2048, C=1024):
  - 8 "short" heads: d_qk=32,  window 256
  - 8 "long"  heads: d_qk=128, window 1024
  - value/output head dim 64, output projection C x C.

Sharding (8 cores): data-parallel over batch (2) x head-parallel over head
groups (4). Core c = 4*b + g handles batch b and heads {2g, 2g+1} of both the
short and long sets. Each core computes its 4 heads' attention plus the
corresponding 256 rows of Wproj, producing a partial [T, C] output; the host
sums the 4 partials per batch element.

Device-side design notes (v2, software-pipelined):
  - scores in f32r (full PE rate at N=512, exact fp32 bits); p/v/Wproj/y in
    bf16 (any-N full rate, 2x DVE modes). Softmax sums kept exact in fp32
    PSUM via a bf16 ones column appended to v.
  - everything computed transposed so no on-device transposes: host passes
    xT [C, T]; projections give qT/kT [d, T] and v [T, dv]; sT[k, q] =
    kT.T @ qT; yT[dv, q] = v_aug.T @ pT.
  - the projection work is software-pipelined into the attention loop:
    projection chunk tch (512 tokens) is emitted between the attention ops
    of query group tch-1, so the PE executes projection matmuls while the
    scalar engine runs exp and the DVE applies band masks. This keeps the
    PE dense (HAM stays un-throttled at 2.4 GHz) instead of stalling on the
    exp->mask->AV chain every head.
  - the output projection of query group g is likewise deferred and emitted
    as PE filler inside query group g+1's head slots.
  - AV matmuls and band-mask multiplies are trimmed to the in-band column
    span of each key block (the first key block of each head stays
    untrimmed so the whole yh PSUM bank is written by the start=True
    matmul before partial-span accumulations land on it).
  - softmax normalization: per head-pair, reciprocal_approx_fast (18-bit,
    5x faster than the iterative divide) of the sums rows, broadcast across
    64 partitions with a single K=2 matmul against a constant selector.
"""

import math

import numpy as np

import concourse.bass as bass
import concourse.mybir as mybir
import concourse.tile as tile
from concourse.bass_utils import run_bass_kernel_spmd

F32 = mybir.dt.float32
F32R = mybir.dt.float32r
BF16 = mybir.dt.bfloat16

B, T, C = 2, 2048, 1024
HS, DS = 8, 32
HL, DL = 8, 128
HD = 64
WIN_S, WIN_L = 256, 1024
NT = T // 128    # 16 t-blocks
NCB = C // 128   # 8 c-blocks
NG = T // 512    # 4 query groups
VW = HD + 1      # v columns + ones column for softmax sums
N_CORES = 8


def _split_waits(nc: bass.Bass) -> int:
    """Walrus in this env accepts at most 1 sync wait per instruction.
    Hoist extra waits onto same-engine InstNoOp instructions placed just
    before the owning instruction (same-engine program order preserves the
    blocking semantics)."""
    import bass_rust

    n_added = 0
    for f in nc.m.functions:
        for bb in f.blocks:
            insts = bb.instructions
            if not any(inst.sync_info and len(inst.sync_info.on_wait) > 1
                       for inst in insts):
                continue
            new = []
            for inst in insts:
                si = inst.sync_info
                waits = list(si.on_wait) if si else []
                if len(waits) > 1:
                    for i, w in enumerate(waits[:-1]):
                        nop = mybir.InstNoOp(
                            name=f"{inst.name}_hw{i}",
                            sync_info=bass_rust.SyncInfo(on_wait=[w], on_update=[]),
                            bass_nofuse=True,
                            engine=inst.engine,
                        )
                        new.append(nop)
                        n_added += 1
                    inst.sync_info = bass_rust.SyncInfo(
                        on_wait=waits[-1:], on_update=list(si.on_update))
                new.append(inst)
            bb.instructions = new
    return n_added


def _patch_tile_drain():
    """This walrus build rejects >1 sync wait on the TileContext tail drain
    ("Too many sync wait commands"). Re-emit the drain's waits as individual
    wait_ge instructions on the sync engine."""
    import bass_rust
    from concourse.tile import ScopedClock, TileContext

    def _drain_and_barrier(self, tick_clock, wait_clock):
        nc = self.nc
        drain_inst = nc.sync.drain()
        wait_clock.add_sem_waits(
            drain_inst.ins, ScopedClock({None: tick_clock.global_clock})
        )
        si = drain_inst.ins.sync_info
        waits = list(si.on_wait) if si is not None else []
        if len(waits) > 1:
            drain_inst.ins.sync_info = bass_rust.SyncInfo(on_wait=[], on_update=[])
            sems = {h.name: h for h in self.sems.allocated().values()}
            for w in waits:
                nc.sync.wait_ge(sems[w.ant_name], w.wait_value)
        nc.all_engine_barrier()
        popped = nc._tile_sem_poison_stack.pop()
        assert popped is self._sem_poison
        nc.clear_and_free_semaphores(list(self.sems.allocated().values()))
        nc.all_engine_barrier()

    TileContext._drain_and_barrier = _drain_and_barrier


_patch_tile_drain()


def _build_program() -> bass.Bass:
    nc = bass.Bass()

    xt_d = nc.dram_tensor("xt", [C, T], BF16, kind="ExternalInput")
    wsqk_d = nc.dram_tensor("wsqk", [C, 128], BF16, kind="ExternalInput")
    wql_d = nc.dram_tensor("wql", [C, 256], BF16, kind="ExternalInput")
    wkl_d = nc.dram_tensor("wkl", [C, 256], BF16, kind="ExternalInput")
    wv_d = nc.dram_tensor("wv", [C, 256], BF16, kind="ExternalInput")
    wp_d = nc.dram_tensor("wp", [256, C], BF16, kind="ExternalInput")
    bs_d = nc.dram_tensor("band_s", [128, WIN_S + 896], BF16, kind="ExternalInput")
    bl_d = nc.dram_tensor("band_l", [128, WIN_L + 896], BF16, kind="ExternalInput")
    ones_d = nc.dram_tensor("ones", [128, 64], BF16, kind="ExternalInput")
    out_d = nc.dram_tensor("out", [T, C], BF16, kind="ExternalOutput")

    scale_s = 1.0 / math.sqrt(DS)
    scale_l = 1.0 / math.sqrt(DL)

    with tile.TileContext(nc) as tc:
        with (
            tc.tile_pool(name="const", bufs=1) as const,
            tc.tile_pool(name="qkp", bufs=1) as qkp,
            tc.tile_pool(name="vp", bufs=1) as vp,
            tc.tile_pool(name="xtp", bufs=1) as xtp,
            tc.tile_pool(name="ptp", bufs=4) as ptp,
            tc.tile_pool(name="ytp", bufs=2) as ytp,
            tc.tile_pool(name="rbsp", bufs=2) as rbsp,
            tc.tile_pool(name="smallp", bufs=2) as smallp,
            tc.tile_pool(name="obp", bufs=3) as obp,
            tc.tile_pool(name="bigps", bufs=2, space="PSUM") as bigps,
            tc.tile_pool(name="p1", bufs=2, space="PSUM") as p1,
        ):
            # ---- DMA order: first projection chunk's dependencies first, so
            # the first matmul starts after ~2.5MB, not ~9MB.
            # DMA issue order = first-use order; each dma_start costs ~0.8us
            # of issue time on the sync queue, so keep the count low.
            wsqk = const.tile([128, NCB, 128], BF16, tag="wsqk", name="wsqk")
            nc.sync.dma_start(wsqk[:], wsqk_d[:, :].rearrange("(cb p) d -> p cb d", p=128))
            xt = [xtp.tile([128, T], BF16, tag=f"xt{cb}", name=f"xt{cb}")
                  for cb in range(NCB)]
            for cb in range(NCB):
                nc.sync.dma_start(
                    xt[cb][:, 0:512], xt_d[cb * 128:(cb + 1) * 128, 0:512])
            wql = const.tile([128, NCB, 256], BF16, tag="wql", name="wql")
            nc.sync.dma_start(wql[:], wql_d[:, :].rearrange("(cb p) d -> p cb d", p=128))
            wkl = const.tile([128, NCB, 256], BF16, tag="wkl", name="wkl")
            nc.sync.dma_start(wkl[:], wkl_d[:, :].rearrange("(cb p) d -> p cb d", p=128))
            wv = const.tile([128, NCB, 256], BF16, tag="wv", name="wv")
            nc.sync.dma_start(wv[:], wv_d[:, :].rearrange("(cb p) d -> p cb d", p=128))
            # bands before the bulk x chunks: qg0's masks need them early
            band_s = const.tile([128, WIN_S + 896], BF16, tag="band_s", name="band_s")
            nc.sync.dma_start(band_s[:], bs_d[:, :])
            band_l = const.tile([128, WIN_L + 896], BF16, tag="band_l", name="band_l")
            nc.sync.dma_start(band_l[:], bl_d[:, :])
            onesb = const.tile([128, 64], BF16, tag="onesb", name="onesb")
            nc.sync.dma_start(onesb[:], ones_d[:, :])
            # x chunk 1 next: qg0's interleaved projection jobs consume it
            for cb in range(NCB):
                nc.sync.dma_start(
                    xt[cb][:, 512:1024], xt_d[cb * 128:(cb + 1) * 128, 512:1024])
            wp0 = const.tile([128, C], BF16, tag="wp0", name="wp0")
            nc.sync.dma_start(wp0[:], wp_d[0:128, :])
            wp1 = const.tile([128, C], BF16, tag="wp1", name="wp1")
            nc.sync.dma_start(wp1[:], wp_d[128:256, :])

            # ---- projection outputs (persist through the whole kernel) ----
            # q/k in bf16: walrus rejects mixed f32r/bf16 matmuls, and a bf16
            # moving operand runs 1 cyc/row at any N, so in-band span
            # trimming of the score matmuls pays.
            qts = qkp.tile([64, T], BF16, tag="qts", name="qts")
            kts = qkp.tile([64, T], BF16, tag="kts", name="kts")
            qtl = [qkp.tile([128, T], BF16, tag=f"qtl{h}", name=f"qtl{h}") for h in range(2)]
            ktl = [qkp.tile([128, T], BF16, tag=f"ktl{h}", name=f"ktl{h}") for h in range(2)]
            # v for all 4 heads in one tile: layout [128, (tb, head, vw)]
            vt = vp.tile([128, NT, 4, VW], BF16, tag="vt", name="vt")
            # ones column of each v block
            nc.sync.dma_start(vt[:, :, :, HD], ones_d[:, 0:4 * NT].rearrange("p (tb i) -> p tb i", i=4))

            # ---- rest of xT, chunk-ordered loads ----
            for tch in range(2, T // 512):
                for cb in range(NCB):
                    csl = (slice(None), slice(tch * 512, (tch + 1) * 512))
                    nc.sync.dma_start(
                        xt[cb][csl],
                        xt_d[cb * 128:(cb + 1) * 128, tch * 512:(tch + 1) * 512])

            # ================= emission helpers =================

            def make_proj_jobs(tch):
                """(qk_jobs, v_jobs): closures, each one PE accumulation job
                + PSUM drain."""
                tsl = slice(tch * 512, (tch + 1) * 512)
                jobs = []

                def qk_job(h, dsts, jidx):
                    def go():
                        ps = bigps.tile([128, 1024], F32, tag="bigps", name="bigps")
                        w = wsqk if h is None else (wql if dsts[0][2] == 'q' else wkl)
                        for cb in range(NCB):
                            lhsT = w[:, cb, :] if h is None else w[:, cb, h * 128:(h + 1) * 128]
                            nc.tensor.matmul(
                                ps[:, 0:512], lhsT, xt[cb][:, tsl],
                                start=(cb == 0), stop=(cb == NCB - 1))
                        with nc.allow_low_precision(reason="bf16 q"):
                            if h is None:
                                nc.vector.tensor_copy(qts[:, tsl], ps[0:64, 0:512])
                                nc.vector.tensor_copy(kts[:, tsl], ps[64:128, 0:512])
                            else:
                                dst = dsts[0][0]
                                nc.vector.tensor_copy(dst[:, tsl], ps[:, 0:512])
                    return go

                jobs.append(qk_job(None, [(None, None, 's')], 0))
                jobs.append(qk_job(0, [(qtl[0], None, 'q')], 1))
                jobs.append(qk_job(0, [(ktl[0], None, 'k')], 2))
                jobs.append(qk_job(1, [(qtl[1], None, 'q')], 3))
                jobs.append(qk_job(1, [(ktl[1], None, 'k')], 4))

                def v_job(tb):
                    def go():
                        ps = bigps.tile([128, 1024], F32, tag="bigps", name="bigps")
                        for cb in range(NCB):
                            nc.tensor.matmul(
                                ps[:, 0:256], xt[cb][:, tb * 128:(tb + 1) * 128], wv[:, cb, :],
                                start=(cb == 0), stop=(cb == NCB - 1))
                        with nc.allow_low_precision(reason="bf16 v"):
                            nc.vector.tensor_copy(
                                vt[:, tb, :, 0:HD],
                                ps[:, 0:256].rearrange("p (i d) -> p i d", d=HD))
                    return go

                vjobs = [v_job(tb) for tb in range(4 * tch, 4 * tch + 4)]
                return jobs, vjobs

            def head_params(qg, hi):
                q0 = qg * 512
                if hi < 2:
                    h = hi
                    return dict(
                        kt_ap=lambda kb, h=h: kts[32 * h: 32 * h + 32, kb * 128:(kb + 1) * 128],
                        qt_ap=qts[32 * h: 32 * h + 32, q0: q0 + 512],
                        win=WIN_S, scale=scale_s, band=band_s,
                    )
                h = hi - 2
                return dict(
                    kt_ap=lambda kb, h=h: ktl[h][:, kb * 128:(kb + 1) * 128],
                    qt_ap=qtl[h][:, q0: q0 + 512],
                    win=WIN_L, scale=scale_l, band=band_l,
                )

            def emit_scores(qg, head):
                """Trimmed score matmuls + exp + trimmed band masks.
                Returns [(kb, pt, jj, a, b)] for the AV stage."""
                p = head_params(qg, head)
                q0 = qg * 512
                win = p['win']
                kb_lo = max(0, q0 - win) // 128
                kb_hi = (q0 + 384) // 128
                kbs = list(range(kb_lo, kb_hi + 1))
                # in-band column span per key block; first kb untrimmed so
                # the start=True AV matmul covers the whole yh bank
                spans = []
                for kb in kbs:
                    delta = kb * 128 - q0
                    first = (kb == kbs[0])
                    a = 0 if first else max(0, delta)
                    b = 512 if first else min(512, delta + win + 128)
                    spans.append((a, b))
                out = []
                for j in range(0, len(kbs), 2):
                    pair = kbs[j: j + 2]
                    st = bigps.tile([128, 1024], F32, tag="bigps", name="bigps")
                    for jj, kb in enumerate(pair):
                        a, b = spans[j + jj]
                        nc.tensor.matmul(
                            st[:, jj * 512 + a: jj * 512 + b],
                            p['kt_ap'](kb), p['qt_ap'][:, a:b],
                            start=True, stop=True)
                    # one exp over the pair's span superset (unwritten gap
                    # columns produce garbage pt values nobody reads)
                    ea = spans[j][0]
                    eb = 512 * (len(pair) - 1) + spans[j + len(pair) - 1][1]
                    pt = ptp.tile([128, 1024], BF16, tag="pt", name="pt")
                    with nc.allow_low_precision(reason="bf16 softmax probs"):
                        nc.scalar.activation(
                            pt[:, ea:eb], st[:, ea:eb],
                            mybir.ActivationFunctionType.Exp, scale=p['scale'])
                    for jj, kb in enumerate(pair):
                        delta = kb * 128 - q0
                        a, b = spans[j + jj]
                        masked = not (512 - win <= delta <= -128)
                        if masked:
                            off = 384 - delta
                            eng = nc.vector if head < 2 else nc.gpsimd
                            psl = (slice(None), slice(jj * 512 + a, jj * 512 + b))
                            with nc.allow_low_precision(reason="bf16 mask"):
                                eng.tensor_tensor(
                                    out=pt[psl], in0=pt[psl],
                                    in1=p['band'][:, off + a: off + b],
                                    op=mybir.AluOpType.mult)
                        out.append((kb, pt, jj, a, b))
                return out

            def emit_av(qg, head, slices, sp, row):
                yh = p1.tile([128, 512], F32, tag="yh", name="yh")
                for i, (kb, pt, jj, a, b) in enumerate(slices):
                    nc.tensor.matmul(
                        yh[0:VW, a:b], vt[:, kb, head, :],
                        pt[:, jj * 512 + a: jj * 512 + b],
                        start=(i == 0), stop=(i == len(slices) - 1))
                # stage the softmax sums row into the pair tile (rows 0/32)
                nc.vector.tensor_copy(sp[row:row + 1, :], yh[HD: HD + 1, :])
                return yh

            def emit_norm(yts_pair, yh_pair, sp):
                # 1/sums as exp(-ln(sums)) on the scalar engine, batched over
                # the head pair (rows 0 and 32; DVE/ACT time only depends on
                # the free size). Both funcs live in one ACT table set.
                lp = smallp.tile([33, 512], F32, tag="ll", name="ll")
                nc.scalar.activation(lp[:, :], sp[:, :],
                                     mybir.ActivationFunctionType.Ln)
                rp = smallp.tile([33, 512], BF16, tag="rr", name="rr")
                with nc.allow_low_precision(reason="bf16 softmax recip"):
                    nc.scalar.activation(rp[:, :], lp[:, :],
                                         mybir.ActivationFunctionType.Exp, scale=-1.0)
                rbs_t = rbsp.tile([128, 512], F32, tag="rbs", name="rbs")
                for k in (0, 1):
                    rb = p1.tile([128, 512], F32, tag="pr", name="pr")
                    nc.tensor.matmul(rb[0:64, :], onesb[32 * k: 32 * k + 1, 0:64],
                                     rp[32 * k: 32 * k + 1, :], start=True, stop=True)
                    nc.vector.tensor_copy(rbs_t[64 * k: 64 * k + 64, :], rb[0:64, :])
                with nc.allow_low_precision(reason="bf16 attn out"):
                    for k in (0, 1):
                        nc.vector.tensor_mul(
                            yts_pair[64 * k: 64 * k + 64, :],
                            yh_pair[k][0:HD, :], rbs_t[64 * k: 64 * k + 64, :])

            def emit_outproj_sub(qg, sub, yts_qg):
                """One 128-query sub-block of query group qg's out-projection."""
                qs = qg * 512 + sub * 128
                ssl = (slice(None), slice(sub * 128, (sub + 1) * 128))
                ob = obp.tile([128, 1024], BF16, tag="ob", name="ob")
                with nc.allow_low_precision(reason="bf16 out"):
                    for nh in range(2):
                        po = p1.tile([128, 512], F32, tag="pr", name="pr")
                        nc.tensor.matmul(po[:, :], yts_qg[0][ssl], wp0[:, nh * 512:(nh + 1) * 512],
                                         start=True, stop=False)
                        nc.tensor.matmul(po[:, :], yts_qg[1][ssl], wp1[:, nh * 512:(nh + 1) * 512],
                                         start=False, stop=True)
                        if nh == 0:
                            nc.vector.tensor_copy(ob[:, 0:512], po[:, :])
                        else:
                            nc.scalar.copy(ob[:, 512:1024], po[:, :])
                nc.sync.dma_start(out_d[qs: qs + 128, :], ob[:])

            # ================= main schedule =================

            # prologue: projection chunk 0
            jq0, jv0 = make_proj_jobs(0)
            for job in jq0 + jv0:
                job()

            HEAD_ORDER = [0, 1, 2, 3]
            prev_yts = None
            deferred_v = []
            for qg in range(NG):
                if qg + 1 < NG:
                    jq, jv = make_proj_jobs(qg + 1)
                    if qg + 1 == NG - 1:
                        # tch3's v blocks are only needed by qg3's AV stage:
                        # keep them as qg3's slot-0 PE filler
                        pjobs, deferred_v = jq, jv
                    else:
                        pjobs = jq + jv
                else:
                    pjobs = deferred_v
                takes = [2, 2, 2, 3] if qg < 2 else ([2, 2, 1, 0] if qg == 2 else [4, 0, 0, 0])
                pj = 0
                yts = [ytp.tile([128, 512], BF16, tag=f"yts{i}", name=f"yts{i}")
                       for i in range(2)]
                yhs = {}
                sp = None
                for slot, head in enumerate(HEAD_ORDER):
                    if slot % 2 == 0:
                        sp = smallp.tile([33, 512], F32, tag="sp", name="sp")
                        nc.gpsimd.memset(sp[:, :], 1.0)
                    slices = emit_scores(qg, head)
                    # PE filler while exp/mask run on scalar/DVE:
                    if prev_yts is not None:
                        emit_outproj_sub(qg - 1, slot, prev_yts)
                    for _ in range(takes[slot]):
                        if pj < len(pjobs):
                            pjobs[pj]()
                            pj += 1
                    yhs[head] = emit_av(qg, head, slices, sp, row=32 * (slot % 2))
                    if slot in (1, 3):
                        pair = HEAD_ORDER[slot - 1: slot + 1]
                        dest = yts[0] if pair[0] < 2 else yts[1]
                        emit_norm(dest, [yhs[pair[0]], yhs[pair[1]]], sp)
                while pj < len(pjobs):
                    pjobs[pj]()
                    pj += 1
                prev_yts = yts

            # epilogue: last query group's out-projection
            for sub in range(4):
                emit_outproj_sub(NG - 1, sub, prev_yts)

    return nc


_PROGRAM = None


def _get_program() -> bass.Bass:
    global _PROGRAM
    if _PROGRAM is None:
        _PROGRAM = _build_program()
        _split_waits(_PROGRAM)
    return _PROGRAM


def _band_image(win: int) -> np.ndarray:
    """[128, win+896] 0/1 image: B[r, u] = 1 iff (u - 384 - r) in [0, win)."""
    u = np.arange(win + 896)[None, :]
    r = np.arange(128)[:, None]
    d = u - 384 - r
    return ((d >= 0) & (d < win)).astype(np.float32)


def make_in_maps(x, Wqk_short, Wv_short, Wqk_long, Wv_long, Wproj):
    """Host-side sharding: per-core input dict for core c = 4*b + g."""
    import ml_dtypes

    bf16 = ml_dtypes.bfloat16
    x = np.asarray(x, dtype=np.float32)
    Wqk_short = np.asarray(Wqk_short, dtype=np.float32).astype(bf16)
    Wv_short = np.asarray(Wv_short, dtype=np.float32).astype(bf16)
    Wqk_long = np.asarray(Wqk_long, dtype=np.float32).astype(bf16)
    Wv_long = np.asarray(Wv_long, dtype=np.float32).astype(bf16)
    Wproj = np.asarray(Wproj, dtype=np.float32)
    assert x.shape == (B, T, C)

    xts = [np.ascontiguousarray(x[b].T.astype(bf16)) for b in range(B)]
    band_s = _band_image(WIN_S).astype(bf16)
    band_l = _band_image(WIN_L).astype(bf16)
    ones = np.ones((128, 64), dtype=bf16)
    in_maps = []
    for c in range(N_CORES):
        b, g = divmod(c, 4)
        wsqk = np.ascontiguousarray(np.concatenate(
            [Wqk_short[:, g * 64:(g + 1) * 64],
             Wqk_short[:, 256 + g * 64: 256 + (g + 1) * 64]], axis=1))
        wql = np.ascontiguousarray(Wqk_long[:, g * 256:(g + 1) * 256])
        wkl = np.ascontiguousarray(Wqk_long[:, 1024 + g * 256: 1024 + (g + 1) * 256])
        wv = np.ascontiguousarray(np.concatenate(
            [Wv_short[:, g * 128:(g + 1) * 128],
             Wv_long[:, g * 128:(g + 1) * 128]], axis=1))
        wp = np.ascontiguousarray(np.concatenate(
            [Wproj[g * 128:(g + 1) * 128, :],
             Wproj[512 + g * 128: 512 + (g + 1) * 128, :]], axis=0).astype(bf16))
        in_maps.append({
            "xt": xts[b], "wsqk": wsqk, "wql": wql, "wkl": wkl, "wv": wv, "wp": wp,
            "band_s": band_s, "band_l": band_l, "ones": ones,
        })
    return in_maps


def gather(results) -> np.ndarray:
    out = np.empty((B, T, C), dtype=np.float32)
    for b in range(B):
        acc = np.zeros((T, C), dtype=np.float64)
        for g in range(4):
            acc += np.asarray(results[4 * b + g]["out"], dtype=np.float32)
        out[b] = acc.astype(np.float32)
    return out


def kernel(x, Wqk_short, Wv_short, Wqk_long, Wv_long, Wproj, **run_kwargs):
    nc = _get_program()
    in_maps = make_in_maps(x, Wqk_short, Wv_short, Wqk_long, Wv_long, Wproj)
    res = run_bass_kernel_spmd(nc, in_maps, core_ids=list(range(N_CORES)), **run_kwargs)
    out = gather(res.results)
    if run_kwargs:
        kernel.last_results = res
    return out


# revision 43
# speedup vs baseline: 1.2121x; 1.1015x over previous
"""Trainium2 Bass kernel for a two-window sparse causal self-attention block.

Model (B=2, T=2048, C=1024):
  - 8 "short" heads: d_qk=32,  window 256
  - 8 "long"  heads: d_qk=128, window 1024
  - value/output head dim 64, output projection C x C.

Sharding (8 cores): data-parallel over batch (2) x head-parallel over head
groups (4). Core c = 4*b + g handles batch b and heads {2g, 2g+1} of both the
short and long sets. Each core computes its 4 heads' attention plus the
corresponding 256 rows of Wproj, producing a partial [T, C] output; the host
sums the 4 partials per batch element.

Device-side design notes (v2, software-pipelined):
  - scores in f32r (full PE rate at N=512, exact fp32 bits); p/v/Wproj/y in
    bf16 (any-N full rate, 2x DVE modes). Softmax sums kept exact in fp32
    PSUM via a bf16 ones column appended to v.
  - everything computed transposed so no on-device transposes: host passes
    xT [C, T]; projections give qT/kT [d, T] and v [T, dv]; sT[k, q] =
    kT.T @ qT; yT[dv, q] = v_aug.T @ pT.
  - the projection work is software-pipelined into the attention loop:
    projection chunk tch (512 tokens) is emitted between the attention ops
    of query group tch-1, so the PE executes projection matmuls while the
    scalar engine runs exp and the DVE applies band masks. This keeps the
    PE dense (HAM stays un-throttled at 2.4 GHz) instead of stalling on the
    exp->mask->AV chain every head.
  - the output projection of query group g is likewise deferred and emitted
    as PE filler inside query group g+1's head slots.
  - AV matmuls and band-mask multiplies are trimmed to the in-band column
    span of each key block (the first key block of each head stays
    untrimmed so the whole yh PSUM bank is written by the start=True
    matmul before partial-span accumulations land on it).
  - softmax normalization: per head-pair, reciprocal_approx_fast (18-bit,
    5x faster than the iterative divide) of the sums rows, broadcast across
    64 partitions with a single K=2 matmul against a constant selector.
"""

import math

import numpy as np

import concourse.bass as bass
import concourse.mybir as mybir
import concourse.tile as tile
from concourse.bass_utils import run_bass_kernel_spmd

F32 = mybir.dt.float32
F32R = mybir.dt.float32r
BF16 = mybir.dt.bfloat16

B, T, C = 2, 2048, 1024
HS, DS = 8, 32
HL, DL = 8, 128
HD = 64
WIN_S, WIN_L = 256, 1024
NT = T // 128    # 16 t-blocks
NCB = C // 128   # 8 c-blocks
NG = T // 512    # 4 query groups
VW = HD + 1      # v columns + ones column for softmax sums
N_CORES = 8


def _split_waits(nc: bass.Bass) -> int:
    """Walrus in this env accepts at most 1 sync wait per instruction.
    Hoist extra waits onto same-engine InstNoOp instructions placed just
    before the owning instruction (same-engine program order preserves the
    blocking semantics)."""
    import bass_rust

    n_added = 0
    for f in nc.m.functions:
        for bb in f.blocks:
            insts = bb.instructions
            if not any(inst.sync_info and len(inst.sync_info.on_wait) > 1
                       for inst in insts):
                continue
            new = []
            for inst in insts:
                si = inst.sync_info
                waits = list(si.on_wait) if si else []
                if len(waits) > 1:
                    for i, w in enumerate(waits[:-1]):
                        nop = mybir.InstNoOp(
                            name=f"{inst.name}_hw{i}",
                            sync_info=bass_rust.SyncInfo(on_wait=[w], on_update=[]),
                            bass_nofuse=True,
                            engine=inst.engine,
                        )
                        new.append(nop)
                        n_added += 1
                    inst.sync_info = bass_rust.SyncInfo(
                        on_wait=waits[-1:], on_update=list(si.on_update))
                new.append(inst)
            bb.instructions = new
    return n_added


def _patch_tile_drain():
    """This walrus build rejects >1 sync wait on the TileContext tail drain
    ("Too many sync wait commands"). Re-emit the drain's waits as individual
    wait_ge instructions on the sync engine."""
    import bass_rust
    from concourse.tile import ScopedClock, TileContext

    def _drain_and_barrier(self, tick_clock, wait_clock):
        nc = self.nc
        drain_inst = nc.sync.drain()
        wait_clock.add_sem_waits(
            drain_inst.ins, ScopedClock({None: tick_clock.global_clock})
        )
        si = drain_inst.ins.sync_info
        waits = list(si.on_wait) if si is not None else []
        if len(waits) > 1:
            drain_inst.ins.sync_info = bass_rust.SyncInfo(on_wait=[], on_update=[])
            sems = {h.name: h for h in self.sems.allocated().values()}
            for w in waits:
                nc.sync.wait_ge(sems[w.ant_name], w.wait_value)
        nc.all_engine_barrier()
        popped = nc._tile_sem_poison_stack.pop()
        assert popped is self._sem_poison
        nc.clear_and_free_semaphores(list(self.sems.allocated().values()))
        nc.all_engine_barrier()

    TileContext._drain_and_barrier = _drain_and_barrier


_patch_tile_drain()


def _build_program() -> bass.Bass:
    nc = bass.Bass()

    xt_d = nc.dram_tensor("xt", [C, T], BF16, kind="ExternalInput")
    wsqk_d = nc.dram_tensor("wsqk", [C, 128], BF16, kind="ExternalInput")
    wql_d = nc.dram_tensor("wql", [C, 256], BF16, kind="ExternalInput")
    wkl_d = nc.dram_tensor("wkl", [C, 256], BF16, kind="ExternalInput")
    wv_d = nc.dram_tensor("wv", [C, 256], BF16, kind="ExternalInput")
    wp_d = nc.dram_tensor("wp", [256, C], BF16, kind="ExternalInput")
    bs_d = nc.dram_tensor("band_s", [128, WIN_S + 896], BF16, kind="ExternalInput")
    bl_d = nc.dram_tensor("band_l", [128, WIN_L + 896], BF16, kind="ExternalInput")
    ones_d = nc.dram_tensor("ones", [128, 64], BF16, kind="ExternalInput")
    out_d = nc.dram_tensor("out", [T, C], BF16, kind="ExternalOutput")

    scale_s = 1.0 / math.sqrt(DS)
    scale_l = 1.0 / math.sqrt(DL)

    with tile.TileContext(nc) as tc:
        with (
            tc.tile_pool(name="const", bufs=1) as const,
            tc.tile_pool(name="qkp", bufs=1) as qkp,
            tc.tile_pool(name="vp", bufs=1) as vp,
            tc.tile_pool(name="xtp", bufs=1) as xtp,
            tc.tile_pool(name="ptp", bufs=8) as ptp,
            tc.tile_pool(name="ytp", bufs=2) as ytp,
            tc.tile_pool(name="rbsp", bufs=2) as rbsp,
            tc.tile_pool(name="smallp", bufs=2) as smallp,
            tc.tile_pool(name="obp", bufs=3) as obp,
            tc.tile_pool(name="bigps", bufs=2, space="PSUM") as bigps,
            tc.tile_pool(name="p1", bufs=2, space="PSUM") as p1,
        ):
            # ---- DMA order: first projection chunk's dependencies first, so
            # the first matmul starts after ~2.5MB, not ~9MB.
            # DMA issue order = first-use order; each dma_start costs ~0.8us
            # of issue time on the sync queue, so keep the count low.
            wsqk = const.tile([128, NCB, 128], BF16, tag="wsqk", name="wsqk")
            nc.sync.dma_start(wsqk[:], wsqk_d[:, :].rearrange("(cb p) d -> p cb d", p=128))
            xt = [xtp.tile([128, T], BF16, tag=f"xt{cb}", name=f"xt{cb}")
                  for cb in range(NCB)]
            for cb in range(NCB):
                nc.sync.dma_start(
                    xt[cb][:, 0:512], xt_d[cb * 128:(cb + 1) * 128, 0:512])
            wql = const.tile([128, NCB, 256], BF16, tag="wql", name="wql")
            wkl = const.tile([128, NCB, 256], BF16, tag="wkl", name="wkl")
            wv = const.tile([128, NCB, 256], BF16, tag="wv", name="wv")
            for w_t, w_d in ((wql, wql_d), (wkl, wkl_d), (wv, wv_d)):
                for half in range(2):
                    cbs = slice(half * 512, (half + 1) * 512)
                    nc.sync.dma_start(
                        w_t[:, half * 4:(half + 1) * 4, :],
                        w_d[cbs, :].rearrange("(cb p) d -> p cb d", p=128))
            # bands before the bulk x chunks: qg0's masks need them early
            band_s = const.tile([128, WIN_S + 896], BF16, tag="band_s", name="band_s")
            nc.sync.dma_start(band_s[:], bs_d[:, :])
            band_l = const.tile([128, WIN_L + 896], BF16, tag="band_l", name="band_l")
            nc.sync.dma_start(band_l[:], bl_d[:, :])
            onesb = const.tile([128, 64], BF16, tag="onesb", name="onesb")
            nc.sync.dma_start(onesb[:], ones_d[:, :])
            # x chunk 1 next: qg0's interleaved projection jobs consume it
            for cb in range(NCB):
                nc.sync.dma_start(
                    xt[cb][:, 512:1024], xt_d[cb * 128:(cb + 1) * 128, 512:1024])
            wp0 = const.tile([128, C], BF16, tag="wp0", name="wp0")
            nc.sync.dma_start(wp0[:], wp_d[0:128, :])
            wp1 = const.tile([128, C], BF16, tag="wp1", name="wp1")
            nc.sync.dma_start(wp1[:], wp_d[128:256, :])

            # ---- projection outputs (persist through the whole kernel) ----
            # q/k in bf16: walrus rejects mixed f32r/bf16 matmuls, and a bf16
            # moving operand runs 1 cyc/row at any N, so in-band span
            # trimming of the score matmuls pays.
            qts = qkp.tile([64, T], BF16, tag="qts", name="qts")
            kts = qkp.tile([64, T], BF16, tag="kts", name="kts")
            qtl = [qkp.tile([128, T], BF16, tag=f"qtl{h}", name=f"qtl{h}") for h in range(2)]
            ktl = [qkp.tile([128, T], BF16, tag=f"ktl{h}", name=f"ktl{h}") for h in range(2)]
            # v for all 4 heads in one tile: layout [128, (tb, head, vw)]
            vt = vp.tile([128, NT, 4, VW], BF16, tag="vt", name="vt")
            # ones column of each v block
            nc.sync.dma_start(vt[:, :, :, HD], ones_d[:, 0:4 * NT].rearrange("p (tb i) -> p tb i", i=4))

            # ---- rest of xT, chunk-ordered loads ----
            for tch in range(2, T // 512):
                for cb in range(NCB):
                    csl = (slice(None), slice(tch * 512, (tch + 1) * 512))
                    nc.sync.dma_start(
                        xt[cb][csl],
                        xt_d[cb * 128:(cb + 1) * 128, tch * 512:(tch + 1) * 512])

            # ================= emission helpers =================

            def make_proj_jobs(tch):
                """(qk_jobs, v_jobs): closures, each one PE accumulation job
                + PSUM drain."""
                tsl = slice(tch * 512, (tch + 1) * 512)
                jobs = []

                def qk_job(h, dsts, jidx):
                    def go():
                        ps = bigps.tile([128, 1024], F32, tag="bigps", name="bigps")
                        w = wsqk if h is None else (wql if dsts[0][2] == 'q' else wkl)
                        for cb in range(NCB):
                            lhsT = w[:, cb, :] if h is None else w[:, cb, h * 128:(h + 1) * 128]
                            nc.tensor.matmul(
                                ps[:, 0:512], lhsT, xt[cb][:, tsl],
                                start=(cb == 0), stop=(cb == NCB - 1))
                        with nc.allow_low_precision(reason="bf16 q"):
                            if h is None:
                                nc.vector.tensor_copy(qts[:, tsl], ps[0:64, 0:512])
                                nc.vector.tensor_copy(kts[:, tsl], ps[64:128, 0:512])
                            else:
                                dst = dsts[0][0]
                                nc.vector.tensor_copy(dst[:, tsl], ps[:, 0:512])
                    return go

                jobs.append(qk_job(None, [(None, None, 's')], 0))
                jobs.append(qk_job(0, [(qtl[0], None, 'q')], 1))
                jobs.append(qk_job(0, [(ktl[0], None, 'k')], 2))
                jobs.append(qk_job(1, [(qtl[1], None, 'q')], 3))
                jobs.append(qk_job(1, [(ktl[1], None, 'k')], 4))

                def v_job(tb):
                    def go():
                        ps = bigps.tile([128, 1024], F32, tag="bigps", name="bigps")
                        for cb in range(NCB):
                            nc.tensor.matmul(
                                ps[:, 0:256], xt[cb][:, tb * 128:(tb + 1) * 128], wv[:, cb, :],
                                start=(cb == 0), stop=(cb == NCB - 1))
                        with nc.allow_low_precision(reason="bf16 v"):
                            nc.vector.tensor_copy(
                                vt[:, tb, :, 0:HD],
                                ps[:, 0:256].rearrange("p (i d) -> p i d", d=HD))
                    return go

                vjobs = [v_job(tb) for tb in range(4 * tch, 4 * tch + 4)]
                return jobs, vjobs

            def head_params(qg, hi):
                q0 = qg * 512
                if hi < 2:
                    h = hi
                    return dict(
                        kt_ap=lambda kb, h=h: kts[32 * h: 32 * h + 32, kb * 128:(kb + 1) * 128],
                        qt_ap=qts[32 * h: 32 * h + 32, q0: q0 + 512],
                        win=WIN_S, scale=scale_s, band=band_s,
                    )
                h = hi - 2
                return dict(
                    kt_ap=lambda kb, h=h: ktl[h][:, kb * 128:(kb + 1) * 128],
                    qt_ap=qtl[h][:, q0: q0 + 512],
                    win=WIN_L, scale=scale_l, band=band_l,
                )

            def emit_scores(qg, head):
                """Trimmed score matmuls + exp + trimmed band masks.
                Returns [(kb, pt, jj, a, b)] for the AV stage."""
                p = head_params(qg, head)
                q0 = qg * 512
                win = p['win']
                kb_lo = max(0, q0 - win) // 128
                kb_hi = (q0 + 384) // 128
                kbs = list(range(kb_lo, kb_hi + 1))
                # in-band column span per key block; first kb untrimmed so
                # the start=True AV matmul covers the whole yh bank
                spans = []
                for kb in kbs:
                    delta = kb * 128 - q0
                    first = (kb == kbs[0])
                    a = 0 if first else max(0, delta)
                    b = 512 if first else min(512, delta + win + 128)
                    spans.append((a, b))
                out = []
                for j in range(0, len(kbs), 2):
                    pair = kbs[j: j + 2]
                    st = bigps.tile([128, 1024], F32, tag="bigps", name="bigps")
                    for jj, kb in enumerate(pair):
                        a, b = spans[j + jj]
                        nc.tensor.matmul(
                            st[:, jj * 512 + a: jj * 512 + b],
                            p['kt_ap'](kb), p['qt_ap'][:, a:b],
                            start=True, stop=True)
                    # one exp over the pair's span superset (unwritten gap
                    # columns produce garbage pt values nobody reads)
                    ea = spans[j][0]
                    eb = 512 * (len(pair) - 1) + spans[j + len(pair) - 1][1]
                    pt = ptp.tile([128, 1024], BF16, tag="pt", name="pt")
                    with nc.allow_low_precision(reason="bf16 softmax probs"):
                        nc.scalar.activation(
                            pt[:, ea:eb], st[:, ea:eb],
                            mybir.ActivationFunctionType.Exp, scale=p['scale'])
                    for jj, kb in enumerate(pair):
                        delta = kb * 128 - q0
                        a, b = spans[j + jj]
                        masked = not (512 - win <= delta <= -128)
                        if masked:
                            off = 384 - delta
                            eng = nc.vector if head < 2 else nc.gpsimd
                            psl = (slice(None), slice(jj * 512 + a, jj * 512 + b))
                            with nc.allow_low_precision(reason="bf16 mask"):
                                eng.tensor_tensor(
                                    out=pt[psl], in0=pt[psl],
                                    in1=p['band'][:, off + a: off + b],
                                    op=mybir.AluOpType.mult)
                        out.append((kb, pt, jj, a, b))
                return out

            def emit_av(qg, head, slices, sp, row):
                yh = p1.tile([128, 512], F32, tag="yh", name="yh")
                for i, (kb, pt, jj, a, b) in enumerate(slices):
                    nc.tensor.matmul(
                        yh[0:VW, a:b], vt[:, kb, head, :],
                        pt[:, jj * 512 + a: jj * 512 + b],
                        start=(i == 0), stop=(i == len(slices) - 1))
                # stage the softmax sums row into the pair tile (rows 0/32)
                nc.vector.tensor_copy(sp[row:row + 1, :], yh[HD: HD + 1, :])
                return yh

            def emit_norm(yts_pair, yh_pair, sp):
                # 1/sums as exp(-ln(sums)) on the scalar engine, batched over
                # the head pair (rows 0 and 32; DVE/ACT time only depends on
                # the free size). Both funcs live in one ACT table set.
                lp = smallp.tile([33, 512], F32, tag="ll", name="ll")
                nc.scalar.activation(lp[:, :], sp[:, :],
                                     mybir.ActivationFunctionType.Ln)
                rp = smallp.tile([33, 512], BF16, tag="rr", name="rr")
                with nc.allow_low_precision(reason="bf16 softmax recip"):
                    nc.scalar.activation(rp[:, :], lp[:, :],
                                         mybir.ActivationFunctionType.Exp, scale=-1.0)
                rbs_t = rbsp.tile([128, 512], F32, tag="rbs", name="rbs")
                for k in (0, 1):
                    rb = p1.tile([128, 512], F32, tag="pr", name="pr")
                    nc.tensor.matmul(rb[0:64, :], onesb[32 * k: 32 * k + 1, 0:64],
                                     rp[32 * k: 32 * k + 1, :], start=True, stop=True)
                    nc.vector.tensor_copy(rbs_t[64 * k: 64 * k + 64, :], rb[0:64, :])
                with nc.allow_low_precision(reason="bf16 attn out"):
                    for k in (0, 1):
                        nc.vector.tensor_mul(
                            yts_pair[64 * k: 64 * k + 64, :],
                            yh_pair[k][0:HD, :], rbs_t[64 * k: 64 * k + 64, :])

            def emit_outproj_sub(qg, sub, yts_qg):
                """One 128-query sub-block of query group qg's out-projection."""
                qs = qg * 512 + sub * 128
                ssl = (slice(None), slice(sub * 128, (sub + 1) * 128))
                ob = obp.tile([128, 1024], BF16, tag="ob", name="ob")
                with nc.allow_low_precision(reason="bf16 out"):
                    for nh in range(2):
                        po = p1.tile([128, 512], F32, tag="pr", name="pr")
                        nc.tensor.matmul(po[:, :], yts_qg[0][ssl], wp0[:, nh * 512:(nh + 1) * 512],
                                         start=True, stop=False)
                        nc.tensor.matmul(po[:, :], yts_qg[1][ssl], wp1[:, nh * 512:(nh + 1) * 512],
                                         start=False, stop=True)
                        if nh == 0:
                            nc.vector.tensor_copy(ob[:, 0:512], po[:, :])
                        else:
                            nc.scalar.copy(ob[:, 512:1024], po[:, :])
                nc.sync.dma_start(out_d[qs: qs + 128, :], ob[:])

            # ================= main schedule =================

            # prologue: projection chunk 0
            jq0, jv0 = make_proj_jobs(0)
            for job in jq0 + jv0:
                job()

            HEAD_ORDER = [0, 1, 2, 3]
            prev_yts = None
            deferred_v = []
            slices_ahead = emit_scores(0, HEAD_ORDER[0])
            for qg in range(NG):
                if qg + 1 < NG:
                    jq, jv = make_proj_jobs(qg + 1)
                    if qg + 1 == NG - 1:
                        # tch3's v blocks are only needed by qg3's AV stage:
                        # keep them as qg3's slot-0 PE filler
                        pjobs, deferred_v = jq, jv
                    else:
                        pjobs = jq + jv
                else:
                    pjobs = deferred_v
                takes = [2, 2, 2, 3] if qg < 2 else ([2, 2, 1, 0] if qg == 2 else [4, 0, 0, 0])
                pj = 0
                yts = [ytp.tile([128, 512], BF16, tag=f"yts{i}", name=f"yts{i}")
                       for i in range(2)]
                yhs = {}
                sp = None
                # software pipeline (crosses query groups): scores run one
                # head ahead of AV, so the PE crunches the next head's scores
                # + filler while this head's exp->mask chain drains, instead
                # of stalling at AV.
                for slot, head in enumerate(HEAD_ORDER):
                    if slot % 2 == 0:
                        sp = smallp.tile([33, 512], F32, tag="sp", name="sp")
                        nc.gpsimd.memset(sp[:, :], 1.0)
                    if slot < 3:
                        nxt_slices = emit_scores(qg, HEAD_ORDER[slot + 1])
                    elif qg + 1 < NG:
                        nxt_slices = emit_scores(qg + 1, HEAD_ORDER[0])
                    else:
                        nxt_slices = None
                    # PE filler while exp/mask run on scalar/DVE:
                    if prev_yts is not None:
                        emit_outproj_sub(qg - 1, slot, prev_yts)
                    for _ in range(takes[slot]):
                        if pj < len(pjobs):
                            pjobs[pj]()
                            pj += 1
                    yhs[head] = emit_av(qg, head, slices_ahead, sp, row=32 * (slot % 2))
                    slices_ahead = nxt_slices
                    if slot in (1, 3):
                        pair = HEAD_ORDER[slot - 1: slot + 1]
                        dest = yts[0] if pair[0] < 2 else yts[1]
                        emit_norm(dest, [yhs[pair[0]], yhs[pair[1]]], sp)
                while pj < len(pjobs):
                    pjobs[pj]()
                    pj += 1
                prev_yts = yts

            # epilogue: last query group's out-projection
            for sub in range(4):
                emit_outproj_sub(NG - 1, sub, prev_yts)

    return nc


_PROGRAM = None


def _get_program() -> bass.Bass:
    global _PROGRAM
    if _PROGRAM is None:
        _PROGRAM = _build_program()
        _split_waits(_PROGRAM)
    return _PROGRAM


def _band_image(win: int) -> np.ndarray:
    """[128, win+896] 0/1 image: B[r, u] = 1 iff (u - 384 - r) in [0, win)."""
    u = np.arange(win + 896)[None, :]
    r = np.arange(128)[:, None]
    d = u - 384 - r
    return ((d >= 0) & (d < win)).astype(np.float32)


def make_in_maps(x, Wqk_short, Wv_short, Wqk_long, Wv_long, Wproj):
    """Host-side sharding: per-core input dict for core c = 4*b + g."""
    import ml_dtypes

    bf16 = ml_dtypes.bfloat16
    x = np.asarray(x, dtype=np.float32)
    Wqk_short = np.asarray(Wqk_short, dtype=np.float32).astype(bf16)
    Wv_short = np.asarray(Wv_short, dtype=np.float32).astype(bf16)
    Wqk_long = np.asarray(Wqk_long, dtype=np.float32).astype(bf16)
    Wv_long = np.asarray(Wv_long, dtype=np.float32).astype(bf16)
    Wproj = np.asarray(Wproj, dtype=np.float32)
    assert x.shape == (B, T, C)

    xts = [np.ascontiguousarray(x[b].T.astype(bf16)) for b in range(B)]
    band_s = _band_image(WIN_S).astype(bf16)
    band_l = _band_image(WIN_L).astype(bf16)
    ones = np.ones((128, 64), dtype=bf16)
    in_maps = []
    for c in range(N_CORES):
        b, g = divmod(c, 4)
        wsqk = np.ascontiguousarray(np.concatenate(
            [Wqk_short[:, g * 64:(g + 1) * 64],
             Wqk_short[:, 256 + g * 64: 256 + (g + 1) * 64]], axis=1))
        wql = np.ascontiguousarray(Wqk_long[:, g * 256:(g + 1) * 256])
        wkl = np.ascontiguousarray(Wqk_long[:, 1024 + g * 256: 1024 + (g + 1) * 256])
        wv = np.ascontiguousarray(np.concatenate(
            [Wv_short[:, g * 128:(g + 1) * 128],
             Wv_long[:, g * 128:(g + 1) * 128]], axis=1))
        wp = np.ascontiguousarray(np.concatenate(
            [Wproj[g * 128:(g + 1) * 128, :],
             Wproj[512 + g * 128: 512 + (g + 1) * 128, :]], axis=0).astype(bf16))
        in_maps.append({
            "xt": xts[b], "wsqk": wsqk, "wql": wql, "wkl": wkl, "wv": wv, "wp": wp,
            "band_s": band_s, "band_l": band_l, "ones": ones,
        })
    return in_maps


def gather(results) -> np.ndarray:
    out = np.empty((B, T, C), dtype=np.float32)
    for b in range(B):
        acc = np.zeros((T, C), dtype=np.float64)
        for g in range(4):
            acc += np.asarray(results[4 * b + g]["out"], dtype=np.float32)
        out[b] = acc.astype(np.float32)
    return out


def kernel(x, Wqk_short, Wv_short, Wqk_long, Wv_long, Wproj, **run_kwargs):
    nc = _get_program()
    in_maps = make_in_maps(x, Wqk_short, Wv_short, Wqk_long, Wv_long, Wproj)
    res = run_bass_kernel_spmd(nc, in_maps, core_ids=list(range(N_CORES)), **run_kwargs)
    out = gather(res.results)
    if run_kwargs:
        kernel.last_results = res
    return out
